# revision 67
# baseline (speedup 1.0000x reference)
"""AnatomyGAT (2-layer RGAT over 1024 graphs) on 8 TRN2 NeuronCores, Bass/Tile.

Sharding: node-parallel. Core c owns nodes [c*6144,(c+1)*6144); edges live on
the dst-owner core, grouped per (dst slot of 128 nodes, relation, src-half)
into 128-edge chunks (SPMD static program; pads use dummy src index 0 and
zero rows in the one-hot M).

Per chunk: transpose dma_gather of h[src] (bf16) -> TensorE per-edge
transform [oj|kj] = h_src @ [W_r|W_r k] -> w = exp(lrelu(qi[dst]+kj)) on the
Activation engine -> U += M^T @ [w*oj | w] in PSUM per slot. Segment softmax
denominator folded in at node level: U/(S+1e-16). h is AllGathered (bf16)
once per layer; per-graph LN stats via dma_scatter_add into a DRAM table +
AllReduce + per-node stats gather by batch id.
"""

import numpy as np
import ml_dtypes

import concourse.bass as bass
import concourse.bacc as bacc
import concourse.mybir as mybir
import concourse.tile as tile
from concourse.bass_utils import run_bass_kernel_spmd

BF16 = ml_dtypes.bfloat16
F32 = mybir.dt.float32
BF = mybir.dt.bfloat16
I16 = mybir.dt.int16

N, G, R, H, C, F = 49152, 1024, 3, 8, 48, 384
NCORES = 8
NS = N // NCORES          # 6144
NSLOT = NS // 128         # 48
NBATCH = 16
BS = NSLOT // NBATCH      # 3
SPLIT = 32767
NEG = 0.2
EPS = 1e-5
GMAX = 7                  # chunks per gather call (HW ring cap ~900 idx/call)
AF = mybir.ActivationFunctionType
ALU = mybir.AluOpType


def _wrap_idx(idx):
    idx = np.asarray(idx, np.int16)
    assert len(idx) % 16 == 0
    return np.tile(idx.reshape(-1, 16).T, (8, 1))


def preprocess(inp):
    f32 = np.float32
    d = {"shared": {}, "percore": [dict() for _ in range(NCORES)]}
    sh = d["shared"]

    # ---- weights ----
    for l, pfx in ((0, "r1"), (1, "r2")):
        W = np.asarray(inp[f"{pfx}_w"], f32)              # [R,384,384]
        q = np.asarray(inp[f"{pfx}_q"], f32)              # [384,8]
        k = np.asarray(inp[f"{pfx}_k"], f32)
        waug = np.concatenate([W, W @ k], axis=2)         # [R,384,392]
        # store [128, kchunk(3), r(3), 392]
        sh[f"waug{l}"] = np.ascontiguousarray(
            waug.reshape(R, 3, 128, 392).transpose(2, 1, 0, 3)
            .reshape(128, 3 * R * 392)).astype(BF16)
        wq = W @ q                                        # [R,384,8]
        sh[f"wq{l}"] = np.ascontiguousarray(
            wq.reshape(R, 3, 128, 8).transpose(2, 1, 0, 3)
            .reshape(128, 3 * R * 8)).astype(BF16)
        rb = np.asarray(inp[f"{pfx}_b"], f32).reshape(1, F)
        sh[f"rb{l}"] = np.repeat(rb, 128, 0).astype(BF16)
        sh[f"nw{l}"] = np.repeat(np.asarray(inp[f"n{l+1}_w"], f32).reshape(1, F),
                                 128, 0).astype(BF16)
        sh[f"nb{l}"] = np.repeat(np.asarray(inp[f"n{l+1}_b"], f32).reshape(1, F),
                                 128, 0).astype(BF16)

    vis_w = np.asarray(inp["vis_w"], f32)                 # [1024,128]
    sh["visw"] = np.ascontiguousarray(
        vis_w.reshape(8, 128, 128).transpose(1, 0, 2).reshape(128, 8 * 128)).astype(BF16)
    encb = np.concatenate([np.asarray(inp["vis_b"], f32),
                           np.asarray(inp["geom_b"], f32),
                           np.asarray(inp["prior_b"], f32)])      # [384]
    # bias folded into spare input rows: xg row 6 = 1 -> gw row 6 = encb[128:256]
    gw = np.zeros((8, 128), f32); gw[:6] = np.asarray(inp["geom_w"], f32)
    gw[6] = encb[128:256]
    sh["gw"] = gw.astype(BF16)
    pw = np.zeros((64, 128), f32); pw[:50] = np.asarray(inp["prior_w"], f32)
    pw[50] = encb[256:384]
    sh["pw"] = pw.astype(BF16)
    sh["encbv"] = encb[0:128].reshape(1, 128).astype(BF16)  # visual region bias
    sh["enclw"] = np.repeat(np.concatenate([np.asarray(inp["vis_lw"], f32),
                                  np.asarray(inp["geom_lw"], f32),
                                  np.asarray(inp["prior_lw"], f32)]).reshape(1, F), 128, 0)
    sh["enclb"] = np.repeat(np.concatenate([np.asarray(inp["vis_lb"], f32),
                                  np.asarray(inp["geom_lb"], f32),
                                  np.asarray(inp["prior_lb"], f32)]).reshape(1, F), 128, 0)
    cw1 = np.asarray(inp["c_w1"], f32)                    # [384,128]
    sh["cw1"] = np.ascontiguousarray(
        cw1.reshape(3, 128, 128).transpose(1, 0, 2).reshape(128, 3 * 128)).astype(BF16)
    sh["identf"] = np.eye(128, dtype=f32)
    sh["cb1"] = np.asarray(inp["c_b1"], f32).reshape(128, 1)
    sh["cw2"] = np.asarray(inp["c_w2"], f32).astype(BF16)
    sh["cb2"] = np.repeat(np.asarray(inp["c_b2"], f32).reshape(1, 49), 128, 0)

    # ---- feature shards ----
    # xvs: per-core [128, NSLOT*1024] with [p, s*1024 + k*128 + n] =
    # xv[base + s*128 + n, k*128 + p] -> slot load is 2KB-contiguous/partition
    xv = np.asarray(inp["x_visual"], f32)
    xg = np.zeros((N, 8), f32); xg[:, :6] = np.asarray(inp["x_geom"], f32)
    xg[:, 6] = 1.0
    xp = np.zeros((N, 64), f32); xp[:, :50] = np.asarray(inp["x_prior"], f32)
    xp[:, 50] = 1.0
    xgT = np.ascontiguousarray(xg.T).astype(BF16)
    xpT = np.ascontiguousarray(xp.T).astype(BF16)
    for c in range(NCORES):
        pc = d["percore"][c]
        blk = xv[c * NS:(c + 1) * NS].reshape(NSLOT, 128, 8, 128)   # s,n,k,p
        pc["xvs"] = np.ascontiguousarray(
            blk.transpose(3, 0, 2, 1).reshape(128, NSLOT * 1024)).astype(BF16)
        pc["xgT"] = np.ascontiguousarray(xgT[:, c * NS:(c + 1) * NS])
        pc["xpT"] = np.ascontiguousarray(xpT[:, c * NS:(c + 1) * NS])

    # ---- edges ----
    srcs, dsts, rels = [], [], []
    for r, key in enumerate(("edge_index_overlap", "edge_index_arch",
                             "edge_index_spatial")):
        e = np.asarray(inp[key], np.int64)
        srcs.append(e[0]); dsts.append(e[1])
        rels.append(np.full(e.shape[1], r, np.int64))
    src = np.concatenate(srcs); dst = np.concatenate(dsts)
    rel = np.concatenate(rels)
    core_of = dst // NS
    slot_of = (dst % NS) // 128
    nrel_of = (dst % 128).astype(np.int64)
    half_of = (src >= SPLIT).astype(np.int64)

    counts = np.zeros((NCORES, NSLOT, R, 2), np.int64)
    np.add.at(counts, (core_of, slot_of, rel, half_of), 1)
    K = -(-counts.max(axis=0) // 128)                     # [NSLOT,R,2]
    K = np.maximum(K, (counts.max(axis=0) > 0).astype(np.int64))

    # bucket edge ids
    keyv = ((core_of * NSLOT + slot_of) * R + rel) * 2 + half_of
    order = np.argsort(keyv, kind="stable")
    sk = keyv[order]
    bounds = np.searchsorted(sk, np.arange(NCORES * NSLOT * R * 2 + 1))

    call_cols = []
    nchunks = int(K.sum())
    for b in range(NBATCH):
        for r in range(R):
            for x in range(2):
                call_cols.append(int(K[b * BS:(b + 1) * BS, r, x].sum()) * 8)
    tot_cols = sum(call_cols)

    for c in range(NCORES):
        eidx = np.zeros((128, tot_cols), np.int16)
        mstream = np.zeros((nchunks, 128, 256), BF16)
        col0 = 0
        ci = 0
        for b in range(NBATCH):
            for r in range(R):
                for x in range(2):
                    ivs = []
                    for si in range(BS):
                        s = b * BS + si
                        kkey = ((c * NSLOT + s) * R + r) * 2 + x
                        es = order[bounds[kkey]:bounds[kkey + 1]]
                        kk = int(K[s, r, x])
                        pad = kk * 128 - len(es)
                        assert pad >= 0
                        sv = src[es] if x == 0 else src[es] - SPLIT
                        ivs.append(np.concatenate([sv, np.zeros(pad, np.int64)]))
                        nr = nrel_of[es]
                        for j in range(kk):
                            lo = j * 128
                            sub = nr[lo:lo + 128]
                            M = np.zeros((128, 128), np.float32)
                            M[np.arange(len(sub)), sub] = 1.0
                            mstream[ci + j, :, :128] = M.astype(BF16)
                            mstream[ci + j, :, 128:] = M.T.astype(BF16)
                        ci += kk
                    if ivs:
                        iv = np.concatenate(ivs)
                        ncols = len(iv) // 16
                        if ncols:
                            eidx[:, col0:col0 + ncols] = _wrap_idx(iv)
                        col0 += ncols
        assert ci == nchunks and col0 == tot_cols, (ci, nchunks, col0, tot_cols)
        d["percore"][c]["eidx"] = eidx
        # [128, nchunks*256]: chunk ci's M|M^T at cols [ci*256,(ci+1)*256)
        d["percore"][c]["mstream"] = np.ascontiguousarray(
            mstream.transpose(1, 0, 2).reshape(128, nchunks * 256))

    # ---- LN graph ----
    # batch is sorted: each core's graphs span < 384 consecutive ids and a
    # graph touches at most 2 adjacent cores. Per-core compact one-hot
    # [node, g - B_c] drives a PSUM matmul accumulation; compact tables are
    # AllGathered and per-node sums recovered by two index gathers.
    batch = np.asarray(inp["batch"], np.int64)
    bc = np.bincount(batch, minlength=G)
    rcnt = (1.0 / (np.maximum(bc, 1) * F)).astype(f32)    # per-graph 1/(n*F)
    Bc = []
    for c in range(NCORES):
        gl = batch[c * NS:(c + 1) * NS]
        B = 128 * (int(gl.min()) // 128)
        assert int(gl.max()) - B <= 382, (c, gl.min(), gl.max(), B)
        Bc.append(B)
    first_node = np.searchsorted(batch, np.arange(G), side="left")
    last_node = np.searchsorted(batch, np.arange(G), side="right") - 1
    c_first = np.clip(first_node // NS, 0, NCORES - 1)
    c_last = np.clip(last_node // NS, 0, NCORES - 1)
    ZROW = 383  # row 383 of core 0's window is always zero (span <= 382)
    for c in range(NCORES):
        gl = batch[c * NS:(c + 1) * NS]
        glm = gl - Bc[c]                                  # [NS] in [0, 383)
        bgc = np.zeros((NSLOT, 128, 384), f32)
        bgc[np.arange(NS) // 128, np.arange(NS) % 128, glm] = 1.0
        d["percore"][c]["bgc"] = np.ascontiguousarray(
            bgc.transpose(1, 0, 2).reshape(128, NSLOT * 384)).astype(BF16)
        ca = c_first[gl]
        cb = c_last[gl]
        idxa = ca * 384 + (gl - np.asarray(Bc)[ca])
        idxb = np.where(cb != ca, cb * 384 + (gl - np.asarray(Bc)[cb]), ZROW)
        d["percore"][c]["gidxa"] = _wrap_idx(idxa)
        d["percore"][c]["gidxb"] = _wrap_idx(idxb)
        d["percore"][c]["rcntn"] = np.ascontiguousarray(
            rcnt[gl].reshape(NSLOT, 128).T)
    sh["sidx"] = _wrap_idx(np.arange(NS))
    d["K"] = K
    d["call_cols"] = call_cols
    d["nchunks"] = nchunks
    d["tot_cols"] = tot_cols
    return d


def build_kernel(pp):
    nc = bacc.Bacc("TRN2", target_bir_lowering=False, debug=False,
                   num_devices=NCORES)
    P = {}

    def param(name, shape, dt):
        P[name] = nc.dram_tensor(name, list(shape), dt, kind="ExternalInput").ap()

    param("xvs", (128, NSLOT * 1024), BF)
    param("xgT", (8, NS), BF); param("xpT", (64, NS), BF)
    param("visw", (128, 8 * 128), BF)
    param("gw", (8, 128), BF); param("pw", (64, 128), BF)
    param("encbv", (1, 128), BF)
    for nm in ("enclw", "enclb"):
        param(nm, (128, F), F32)
    for l in range(2):
        param(f"waug{l}", (128, 3 * R * 392), BF)
        param(f"wq{l}", (128, 3 * R * 8), BF)
        for nm in (f"rb{l}", f"nw{l}", f"nb{l}"):
            param(nm, (128, F), BF)
    param("cw1", (128, 3 * 128), BF); param("cb1", (128, 1), F32)
    param("identf", (128, 128), F32)
    param("cw2", (128, 49), BF); param("cb2", (128, 49), F32)
    param("eidx", (128, pp["tot_cols"]), I16)
    param("mstream", (128, pp["nchunks"] * 256), BF)
    param("gidxa", (128, NS // 16), I16)
    param("gidxb", (128, NS // 16), I16)
    param("sidx", (128, NS // 16), I16)
    param("rcntn", (128, NSLOT), F32)
    param("bgc", (128, NSLOT * 384), BF)
    out_p = nc.dram_tensor("out", [NS, 49], F32, kind="ExternalOutput").ap()

    K = pp["K"]; call_cols = pp["call_cols"]
    rg_all = [list(range(NCORES))]

    with tile.TileContext(nc) as tc:
        with (
            tc.tile_pool(name="const", bufs=1) as cpool,
            tc.tile_pool(name="slab", bufs=1) as slab,
            tc.tile_pool(name="work", bufs=3) as work,
            tc.tile_pool(name="gep", bufs=4) as gep,
            tc.tile_pool(name="xvp", bufs=2) as xvp,
            tc.tile_pool(name="bgp", bufs=1) as bgp,
            tc.tile_pool(name="htp", bufs=2) as htp,
            tc.tile_pool(name="sgp", bufs=2) as sgp,
            tc.tile_pool(name="mp", bufs=4) as mpool,
            tc.tile_pool(name="ps", bufs=1, space="PSUM") as pspool,
            tc.tile_pool(name="pst", bufs=4, space="PSUM") as pstmp,
            tc.tile_pool(name="dram", bufs=1, space="DRAM") as dpool,
        ):
            # ---- resident consts ----
            cons = {}
            for nm, cols, dt, prows in (
                ("visw", 8 * 128, BF, 128), ("gw", 128, BF, 8), ("pw", 128, BF, 64),
                ("encbv", 128, BF, 1),
                ("enclw", F, F32, 128), ("enclb", F, F32, 128),
                ("waug0", 3 * R * 392, BF, 128), ("wq0", 3 * R * 8, BF, 128),
                ("waug1", 3 * R * 392, BF, 128), ("wq1", 3 * R * 8, BF, 128),
                ("rb0", F, BF, 128), ("nw0", F, BF, 128), ("nb0", F, BF, 128),
                ("rb1", F, BF, 128), ("nw1", F, BF, 128), ("nb1", F, BF, 128),
                ("cw1", 3 * 128, BF, 128), ("cb1", 1, F32, 128),
                ("identf", 128, F32, 128),
                ("cw2", 49, BF, 128), ("cb2", 49, F32, 128),
                ("eidx", pp["tot_cols"], I16, 128),
                ("gidxa", NS // 16, I16, 128), ("gidxb", NS // 16, I16, 128),
                ("sidx", NS // 16, I16, 128),
                ("rcntn", NSLOT, F32, 128),
            ):
                t = cpool.tile([prows if prows > 1 else 1, cols], dt, tag=nm)
                nc.sync.dma_start(out=t[:prows, :], in_=P[nm][:])
                cons[nm] = t
            waugv = [cons[f"waug{l}"].rearrange("p (k r w) -> p k r w", k=3, r=R)
                     for l in range(2)]
            viswv = cons["visw"].rearrange("p (k f) -> p k f", k=8)
            cw1v = cons["cw1"].rearrange("p (k f) -> p k f", k=3)
            mstv = P["mstream"].rearrange("p (ci e) -> p ci e", e=256)

            h_slab = slab.tile([128, NSLOT * F], BF, tag="h")
            hs = h_slab.rearrange("p (s f) -> p s f", s=NSLOT)
            # per-node [sum, sumsq] accumulator for graph-LN scatter
            rs_slab = slab.tile([128, NSLOT * 2], F32, tag="rs")
            rsv = rs_slab.rearrange("p (s e) -> p s e", s=NSLOT)
            o_slab = slab.tile([128, NSLOT * 49], F32, tag="o")
            osv = o_slab.rearrange("p (s e) -> p s e", s=NSLOT)

            h_local = dpool.tile([NS, F], BF, tag="hl")
            h_all = dpool.tile([N, F], BF, tag="ha")
            enc_b1 = dpool.tile([1, 8], F32, tag="eb1")
            enc_b2 = dpool.tile([1, 8], F32, tag="eb2")
            g_loc = dpool.tile([384, 64], F32, tag="gl")
            g_all = dpool.tile([NCORES * 384, 64], F32, tag="ga")

            onesbf = cpool.tile([1, 128], BF, tag="onesbf")
            nc.vector.memset(onesbf[:1, :], 1.0)
            zero64 = cpool.tile([128, 3 * 64], F32, tag="z64")
            nc.vector.memset(zero64[:], 0.0)

            # ================= encoders =================
            sum6 = slab.tile([128, 6], F32, tag="s6")
            nc.vector.memset(sum6[:], 0.0)
            xvsv = P["xvs"].rearrange("p (s e) -> p s e", s=NSLOT)
            # geom/prior features resident (small)
            xgall = slab.tile([8, NS], BF, tag="xgall")
            nc.sync.dma_start(out=xgall[:8, :], in_=P["xgT"][:])
            xpall = slab.tile([64, NS], BF, tag="xpall")
            nc.sync.dma_start(out=xpall[:64, :], in_=P["xpT"][:])
            for s4 in range(NSLOT // 4):
                xvt4 = xvp.tile([128, 4 * 8 * 128], BF, tag="xv")
                nc.sync.dma_start(
                    out=xvt4.rearrange("p (s e) -> p s e", s=4)[:],
                    in_=xvsv[:, s4 * 4:(s4 + 1) * 4, :])
                xvt4v = xvt4.rearrange("p (s k n) -> p s k n", s=4, k=8)
                for s4i in range(4):
                    s = s4 * 4 + s4i
                    ps = pstmp.tile([128, 400], F32, tag="pt")
                    for kk in range(8):
                        nc.tensor.matmul(out=ps[:, 0:128],
                                         lhsT=xvt4v[:, s4i, kk, :],
                                         rhs=viswv[:, kk, :],
                                         start=(kk == 0), stop=False)
                    nc.tensor.matmul(out=ps[:, 0:128], lhsT=onesbf[:1, :],
                                     rhs=cons["encbv"][:1, :], start=False, stop=True)
                    nc.tensor.matmul(out=ps[:, 128:256],
                                     lhsT=xgall[:8, bass.ts(s, 128)],
                                     rhs=cons["gw"][:8, :], start=True, stop=True)
                    nc.tensor.matmul(out=ps[:, 256:384],
                                     lhsT=xpall[:64, bass.ts(s, 128)],
                                     rhs=cons["pw"][:64, :], start=True, stop=True)
                    # relu straight into the h slab (bf16)
                    nc.scalar.activation(out=hs[:, s, :], in_=ps[:, 0:384],
                                         func=AF.Relu)
                    sq = work.tile([128, F], BF, tag="sq")
                    nc.vector.tensor_tensor(out=sq[:], in0=hs[:, s, :],
                                            in1=hs[:, s, :], op=ALU.mult)
                    r1 = work.tile([128, 3], F32, tag="r1")
                    r2 = work.tile([128, 3], F32, tag="r2")
                    nc.vector.tensor_reduce(out=r1[:],
                                            in_=hs[:, s, :].rearrange(
                                                "p (b f) -> p b f", b=3)[:],
                                            axis=mybir.AxisListType.X, op=ALU.add)
                    nc.vector.tensor_reduce(out=r2[:],
                                            in_=sq.rearrange(
                                                "p (b f) -> p b f", b=3)[:],
                                            axis=mybir.AxisListType.X, op=ALU.add)
                    nc.vector.tensor_tensor(out=sum6[:, 0:3], in0=sum6[:, 0:3],
                                            in1=r1[:], op=ALU.add)
                    nc.vector.tensor_tensor(out=sum6[:, 3:6], in0=sum6[:, 3:6],
                                            in1=r2[:], op=ALU.add)
            ps6 = pstmp.tile([6, 1], F32, tag="pt")
            onesf = cpool.tile([128, 1], F32, tag="onesf")
            nc.vector.memset(onesf[:], 1.0)
            nc.tensor.matmul(out=ps6[:], lhsT=sum6[:], rhs=onesf[:],
                             start=True, stop=True)
            s6s = work.tile([8, 1], F32, tag="s6s")
            nc.vector.memset(s6s[:8, :], 0.0)
            nc.vector.tensor_copy(out=s6s[:6, :], in_=ps6[:])
            nc.gpsimd.dma_start(out=enc_b1[0, 0:8], in_=s6s[:8, 0])
            nc.gpsimd.collective_compute("AllReduce", ALU.add,
                                         replica_groups=rg_all,
                                         ins=[enc_b1.opt()], outs=[enc_b2.opt()])
            es1 = work.tile([1, 8], F32, tag="es")
            nc.sync.dma_start(out=es1[:1, :], in_=enc_b2[:])
            ones1 = cpool.tile([1, 128], F32, tag="ones1")
            nc.vector.memset(ones1[:1, :], 1.0)
            psb = pstmp.tile([128, 400], F32, tag="pt")
            nc.tensor.matmul(out=psb[:, 0:8], lhsT=ones1[:1, :], rhs=es1[:1, :],
                             start=True, stop=True)
            es = work.tile([128, 8], F32, tag="esb")
            nc.vector.tensor_copy(out=es[:], in_=psb[:, 0:8])
            cntE = float(N * 128)
            m3 = work.tile([128, 8], F32, tag="m3")
            nc.vector.tensor_scalar_mul(m3[:, 0:3], es[:, 0:3], 1.0 / cntE)
            v3 = work.tile([128, 8], F32, tag="v3")
            nc.vector.tensor_scalar_mul(v3[:, 0:3], es[:, 3:6], 1.0 / cntE)
            q3 = work.tile([128, 8], F32, tag="q3")
            nc.vector.tensor_tensor(out=q3[:, 0:3], in0=m3[:, 0:3],
                                    in1=m3[:, 0:3], op=ALU.mult)
            nc.vector.tensor_tensor(out=v3[:, 0:3], in0=v3[:, 0:3],
                                    in1=q3[:, 0:3], op=ALU.subtract)
            nc.scalar.activation(out=v3[:, 0:3], in_=v3[:, 0:3], func=AF.Sqrt)
            nc.vector.tensor_scalar_add(v3[:, 0:3], v3[:, 0:3], EPS)
            nc.vector.reciprocal(out=v3[:, 0:3], in_=v3[:, 0:3])
            c1 = slab.tile([128, F], BF, tag="c1")
            c0 = slab.tile([128, F], BF, tag="c0")
            c1f = work.tile([128, F], F32, tag="c1f")
            nc.vector.tensor_tensor(
                out=c1f.rearrange("o (b f) -> o b f", b=3)[:],
                in0=cons["enclw"].rearrange("o (b f) -> o b f", b=3)[:],
                in1=v3[:, 0:3].to_broadcast([128, 3, 128]), op=ALU.mult)
            c0f = work.tile([128, F], F32, tag="c0f")
            nc.vector.tensor_tensor(
                out=c0f.rearrange("o (b f) -> o b f", b=3)[:],
                in0=c1f.rearrange("o (b f) -> o b f", b=3)[:],
                in1=m3[:, 0:3].to_broadcast([128, 3, 128]), op=ALU.mult)
            nc.vector.tensor_tensor(out=c0f[:], in0=cons["enclb"][:],
                                    in1=c0f[:], op=ALU.subtract)
            nc.vector.tensor_copy(out=c1[:], in_=c1f[:])
            nc.vector.tensor_copy(out=c0[:], in_=c0f[:])
            for g8 in range(8):
                for si in range(6):
                    s = g8 * 6 + si
                    nc.vector.tensor_tensor(out=hs[:, s, :], in0=hs[:, s, :],
                                            in1=c1[:], op=ALU.mult)
                    nc.vector.tensor_tensor(out=hs[:, s, :], in0=hs[:, s, :],
                                            in1=c0[:], op=ALU.add)
                nc.sync.dma_start(
                    out=h_local.rearrange("(s p) f -> p s f", p=128)[
                        :, g8 * 6:(g8 + 1) * 6, :],
                    in_=hs[:, g8 * 6:(g8 + 1) * 6, :])

            # ================= RGAT layers =================
            for l in range(2):
                nc.gpsimd.collective_compute("AllGather", ALU.bypass,
                                             replica_groups=rg_all,
                                             ins=[h_local.opt()], outs=[h_all.opt()])
                # zero cols 2:64 of the compact graph-stats table (the stats
                # gather reads whole 64-col rows)
                nc.sync.dma_start(
                    out=g_loc.rearrange("(k p) e -> p k e", p=128)[:, :, 2:64],
                    in_=zero64.rearrange("p (j e) -> p j e", j=3)[:, :, 2:64])
                # compact graph-stats accumulator (SBUF)
                gacc = slab.tile([128, 6], F32, tag="gacc")
                nc.vector.memset(gacc[:], 0.0)
                # prelude: qi for all 48 slots (needs only h_local -> runs
                # during the AllGather)
                qslab = slab.tile([128, NSLOT * R * 8], BF, tag="qslab")
                qsv = qslab.rearrange("p (s r h) -> p s r h", s=NSLOT, r=R)
                for g8 in range(8):
                    hts = htp.tile([128, 3 * 6 * 128], BF, tag="hts")
                    htsv = hts.rearrange("p (k e) -> p k e", k=3)
                    nc.gpsimd.dma_gather(
                        out_ap=htsv[:], in_ap=h_local[:],
                        idxs_ap=cons["sidx"][:, g8 * 6 * 8:(g8 + 1) * 6 * 8],
                        num_idxs=6 * 128, num_idxs_reg=6 * 128,
                        elem_size=F, transpose=True)
                    for si in range(6):
                        s = g8 * 6 + si
                        pq = pstmp.tile([128, 400], F32, tag="pt")
                        for kk in range(3):
                            nc.tensor.matmul(
                                out=pq[:, 0:R * 8],
                                lhsT=htsv[:, kk, bass.ts(si, 128)],
                                rhs=cons[f"wq{l}"][:, kk * R * 8:(kk + 1) * R * 8],
                                start=(kk == 0), stop=(kk == 2))
                        nc.vector.tensor_copy(
                            out=qsv[:, s, :, :],
                            in_=pq[:, 0:R * 8].rearrange("p (r h) -> p r h", r=R)[:])
                ci = 0
                col0 = 0
                cci = 0
                for b in range(NBATCH):
                    ges = {}   # (r, x) -> (list of views, list of M views)
                    for r in range(R):
                        for x in range(2):
                            S16 = call_cols[cci]; cci += 1
                            S = S16 * 16
                            if S == 0:
                                continue
                            nch = S // 128
                            subs = []
                            msubs = []
                            for g0 in range(0, nch, GMAX):
                                gn = min(GMAX, nch - g0)
                                Ssub = gn * 128
                                ge = gep.tile([128, 3 * Ssub], BF, tag="ge",
                                              name=f"ge{r}_{x}_{g0}")
                                src_view = (h_all[0:SPLIT + 1, :] if x == 0
                                            else h_all[SPLIT:N, :])
                                nc.gpsimd.dma_gather(
                                    out_ap=ge.rearrange("p (k e) -> p k e", k=3)[:],
                                    in_ap=src_view,
                                    idxs_ap=cons["eidx"][:, col0 + g0 * 8:
                                                         col0 + g0 * 8 + Ssub // 16],
                                    num_idxs=Ssub, num_idxs_reg=Ssub,
                                    elem_size=F, transpose=True)
                                subs.append(ge.rearrange("p (k e) -> p k e", k=3))
                                # grouped M|M^T load for these chunks
                                mg = mpool.tile([128, gn * 256], BF, tag="mp")
                                nc.sync.dma_start(
                                    out=mg[:],
                                    in_=P["mstream"][:, (ci + g0) * 256:
                                                     (ci + g0 + gn) * 256])
                                msubs.append(mg.rearrange("p (c e) -> p c e", c=gn))
                            ges[(r, x)] = (subs, msubs)
                            ci += nch
                            col0 += S16
                    upb = []
                    for si in range(BS):
                        ut = pspool.tile([128, 400], F32, tag=f"u{si}", name=f"u{b}_{si}")
                        upb.append(ut)
                    started = [False] * BS
                    # last (r, x) group with chunks, per slot (to set stop=)
                    last_rx = {}
                    for si in range(BS):
                        for r in range(R):
                            for x in range(2):
                                if int(K[b * BS + si, r, x]) > 0:
                                    last_rx[si] = (r, x)
                    for r in range(R):
                        for x in range(2):
                            subs_m = ges.get((r, x))
                            if subs_m is None:
                                continue
                            subs, msubs = subs_m
                            cl = 0
                            for si in range(BS):
                                s = b * BS + si
                                for j in range(int(K[s, r, x])):
                                    gev = subs[cl // GMAX]
                                    mgv = msubs[cl // GMAX]
                                    off = (cl % GMAX) * 128
                                    mslice = mgv[:, cl % GMAX, :]
                                    cl += 1
                                    pt = pstmp.tile([128, 400], F32, tag="pt")
                                    for kk in range(2):
                                        nc.tensor.matmul(
                                            out=pt[:, 0:392],
                                            lhsT=gev[:, kk, off:off + 128],
                                            rhs=waugv[l][:, kk, r, :],
                                            start=(kk == 0), stop=False)
                                    # qi[dst] accumulates onto kj in PSUM
                                    # (mid-group; last transform closes it)
                                    nc.tensor.matmul(
                                        out=pt[:, 384:392], lhsT=mslice[:, 128:256],
                                        rhs=qsv[:, s, r, :], start=False,
                                        stop=False, skip_group_check=True)
                                    nc.tensor.matmul(
                                        out=pt[:, 0:392],
                                        lhsT=gev[:, 2, off:off + 128],
                                        rhs=waugv[l][:, 2, r, :],
                                        start=False, stop=True)
                                    # w = exp(lrelu(a)) = max(exp(a), exp(NEG*a))
                                    me = work.tile([128, 392], BF, tag="me")
                                    at = work.tile([128, 8], F32, tag="at")
                                    nc.scalar.activation(out=at[:],
                                                         in_=pt[:, 384:392],
                                                         func=AF.Exp)
                                    nc.scalar.activation(out=me[:, 384:392],
                                                         in_=pt[:, 384:392],
                                                         func=AF.Exp, scale=NEG)
                                    nc.vector.tensor_tensor(out=me[:, 384:392],
                                                            in0=at[:],
                                                            in1=me[:, 384:392],
                                                            op=ALU.max)
                                    nc.vector.tensor_tensor(
                                        out=me[:, 0:384].rearrange(
                                            "p (h c) -> p h c", h=H)[:],
                                        in0=pt[:, 0:384].rearrange(
                                            "p (h c) -> p h c", h=H)[:],
                                        in1=me[:, 384:392].to_broadcast([128, H, C]),
                                        op=ALU.mult)
                                    is_last = (last_rx.get(si) == (r, x)
                                               and j == int(K[s, r, x]) - 1)
                                    nc.tensor.matmul(
                                        out=upb[si][:, 0:392], lhsT=mslice[:, 0:128],
                                        rhs=me[:], start=not started[si],
                                        stop=is_last)
                                    started[si] = True
                    for si in range(BS):
                        s = b * BS + si
                        up = upb[si]
                        if not started[si]:
                            nc.vector.memset(up[:], 0.0)
                        sr = work.tile([128, 8], F32, tag="sr")
                        nc.vector.tensor_scalar_add(sr[:], up[:, 384:392], 1e-16)
                        nc.vector.reciprocal(out=sr[:], in_=sr[:])
                        z = work.tile([128, F], BF, tag="z")
                        nc.vector.tensor_tensor(
                            out=z.rearrange("p (h c) -> p h c", h=H)[:],
                            in0=up[:, 0:384].rearrange("p (h c) -> p h c", h=H)[:],
                            in1=sr[:].to_broadcast([128, H, C]), op=ALU.mult)
                        nc.vector.tensor_tensor(out=z[:], in0=z[:],
                                                in1=cons[f"rb{l}"][:], op=ALU.add)
                        # elu(z) = max(z, exp(min(z,0)) - 1);
                        # min(z,0) = -relu(-z), both on the Act engine
                        e1 = work.tile([128, F], BF, tag="e1")
                        nc.scalar.activation(out=e1[:], in_=z[:], func=AF.Relu,
                                             scale=-1.0)
                        nc.scalar.activation(out=e1[:], in_=e1[:], func=AF.Exp,
                                             scale=-1.0)
                        # fused: m = max(e1 - 1, z); h += m (accum -> sum);
                        # sq = h*h (accum -> sumsq)
                        nc.vector.scalar_tensor_tensor(
                            out=e1[:], in0=e1[:], scalar=-1.0, in1=z[:],
                            op0=ALU.add, op1=ALU.max)
                        nc.vector.scalar_tensor_tensor(
                            out=hs[:, s, :], in0=hs[:, s, :], scalar=0.0,
                            in1=e1[:], op0=ALU.add, op1=ALU.add,
                            accum_out=rsv[:, s, 0:1])
                        sq2 = work.tile([128, F], BF, tag="sq2")
                        nc.vector.scalar_tensor_tensor(
                            out=sq2[:], in0=hs[:, s, :], scalar=0.0,
                            in1=hs[:, s, :], op0=ALU.add, op1=ALU.mult,
                            accum_out=rsv[:, s, 1:2])
                        # accumulate per-graph [sum, sumsq] via compact
                        # one-hot matmul (window [B_c, B_c+384))
                        rsb = work.tile([128, 2], BF, tag="rsb")
                        nc.vector.tensor_copy(out=rsb[:], in_=rsv[:, s, :])
                        if s % 6 == 0:
                            bgt = bgp.tile([128, 6 * 384], BF, tag="bgt")
                            nc.sync.dma_start(
                                out=bgt[:],
                                in_=P["bgc"][:, s * 384:(s + 6) * 384])
                            bgv = bgt.rearrange("p (w k e) -> p w k e",
                                                w=6, k=3)
                        gps = pstmp.tile([128, 400], F32, tag="pt")
                        for kk in range(3):
                            nc.tensor.matmul(
                                out=gps[:, 384 + 2 * kk:386 + 2 * kk],
                                lhsT=bgv[:, s % 6, kk, :], rhs=rsb[:],
                                start=True, stop=True)
                        nc.vector.tensor_tensor(out=gacc[:], in0=gacc[:],
                                                in1=gps[:, 384:390],
                                                op=ALU.add)
                # publish compact stats, AllGather all cores' tables
                nc.sync.dma_start(
                    out=g_loc.rearrange("(k p) e -> p k e", p=128)[:, :, 0:2],
                    in_=gacc.rearrange("p (k e) -> p k e", k=3)[:])
                nc.gpsimd.collective_compute("AllGather", ALU.bypass,
                                             replica_groups=rg_all,
                                             ins=[g_loc.opt()], outs=[g_all.opt()])
                for gb in range(8):
                    stga = sgp.tile([128, 6 * 64], F32, tag="stga")
                    stgav = stga.rearrange("p (s e) -> p s e", s=6)
                    stgb = sgp.tile([128, 6 * 64], F32, tag="stgb")
                    stgbv = stgb.rearrange("p (s e) -> p s e", s=6)
                    for idx_t, ov in (("gidxa", stgav), ("gidxb", stgbv)):
                        nc.gpsimd.dma_gather(
                            out_ap=ov[:],
                            in_ap=g_all[:],
                            idxs_ap=cons[idx_t][:, gb * 6 * 8:(gb + 1) * 6 * 8],
                            num_idxs=6 * 128, num_idxs_reg=6 * 128,
                            elem_size=64, transpose=False)
                    stgv = work.tile([128, 6 * 2], F32, tag="stg2")
                    stgv = stgv.rearrange("p (s e) -> p s e", s=6)
                    nc.vector.tensor_tensor(out=stgv[:], in0=stgav[:, :, 0:2],
                                            in1=stgbv[:, :, 0:2], op=ALU.add)
                    # per-node mean / inv-std for this group of slots
                    mn = work.tile([128, 6], F32, tag="mn")
                    nc.vector.tensor_tensor(out=mn[:], in0=stgv[:, :, 0],
                                            in1=cons["rcntn"][:, gb * 6:(gb + 1) * 6],
                                            op=ALU.mult)
                    iv = work.tile([128, 6], F32, tag="iv")
                    nc.vector.tensor_tensor(out=iv[:], in0=stgv[:, :, 1],
                                            in1=cons["rcntn"][:, gb * 6:(gb + 1) * 6],
                                            op=ALU.mult)
                    m2 = work.tile([128, 6], F32, tag="m2")
                    nc.vector.tensor_tensor(out=m2[:], in0=mn[:], in1=mn[:],
                                            op=ALU.mult)
                    nc.vector.tensor_tensor(out=iv[:], in0=iv[:], in1=m2[:],
                                            op=ALU.subtract)
                    nc.vector.tensor_scalar_add(iv[:], iv[:], EPS)
                    nc.scalar.activation(out=iv[:], in_=iv[:], func=AF.Sqrt)
                    nc.vector.reciprocal(out=iv[:], in_=iv[:])
                    for si in range(6):
                        s = gb * 6 + si
                        t = work.tile([128, F], BF, tag="lt")
                        nc.vector.tensor_scalar(
                            out=t[:], in0=hs[:, s, :],
                            scalar1=mn[:, si:si + 1], scalar2=iv[:, si:si + 1],
                            op0=ALU.subtract, op1=ALU.mult)
                        nc.vector.tensor_tensor(out=t[:], in0=t[:],
                                                in1=cons[f"nw{l}"][:], op=ALU.mult)
                        if l == 0:
                            nc.vector.tensor_tensor(out=hs[:, s, :], in0=t[:],
                                                    in1=cons[f"nb{l}"][:],
                                                    op=ALU.add)
                        if l == 1:
                            # fused head: LN result lands directly in the f32
                            # tile the PE transpose reads (hs not needed again)
                            hf = work.tile([128, F], F32, tag="hf")
                            nc.vector.tensor_tensor(out=hf[:], in0=t[:],
                                                    in1=cons[f"nb{l}"][:],
                                                    op=ALU.add)
                            tp = pstmp.tile([128, 400], F32, tag="pt")
                            for kk in range(3):
                                nc.tensor.transpose(
                                    out=tp[:, kk * 128:(kk + 1) * 128],
                                    in_=hf[:, bass.ts(kk, 128)],
                                    identity=cons["identf"][:])
                            htt = work.tile([128, F], BF, tag="htt")
                            nc.scalar.activation(out=htt[:], in_=tp[:, 0:384],
                                                 func=AF.Copy)
                            pz = pstmp.tile([128, 400], F32, tag="pt")
                            for kk in range(3):
                                nc.tensor.matmul(out=pz[:, 0:128],
                                                 lhsT=cw1v[:, kk, :],
                                                 rhs=htt[:, bass.ts(kk, 128)],
                                                 start=(kk == 0), stop=(kk == 2))
                            z1 = work.tile([128, 128], BF, tag="z1")
                            nc.scalar.activation(out=z1[:], in_=pz[:, 0:128],
                                                 func=AF.Relu, bias=cons["cb1"][:])
                            nc.tensor.matmul(out=pz[:, 256:305], lhsT=z1[:],
                                             rhs=cons["cw2"][:],
                                             start=True, stop=True)
                            nc.vector.tensor_tensor(
                                out=osv[:, s, :], in0=pz[:, 256:305],
                                in1=cons["cb2"][:], op=ALU.add)
                    if l == 0:
                        nc.sync.dma_start(
                            out=h_local.rearrange("(s p) f -> p s f", p=128)[
                                :, gb * 6:(gb + 1) * 6, :],
                            in_=hs[:, gb * 6:(gb + 1) * 6, :])

            # ================= output =================
            nc.sync.dma_start(
                out=out_p.rearrange("(s p) f -> p s f", p=128)[:],
                in_=osv[:])

    nc.compile()
    return nc


def kernel(**inputs):
    pp = preprocess(inputs)
    nc = build_kernel(pp)
    in_maps = []
    for c in range(NCORES):
        m = dict(pp["shared"])
        m.update(pp["percore"][c])
        in_maps.append(m)
    res = run_bass_kernel_spmd(nc, in_maps, core_ids=list(range(NCORES)))
    out = np.concatenate([res.results[c]["out"] for c in range(NCORES)], axis=0)
    return out.astype(np.float32)


if __name__ == "__main__":
    import time
    import jax
    import reference
    t0 = time.perf_counter()
    with jax.default_device(jax.devices("cpu")[0]):
        inputs = {k: np.asarray(v) for k, v in reference.setup_inputs().items()}
        exp = np.asarray(reference.reference(**inputs))
    print(f"reference done in {time.perf_counter()-t0:.1f}s")
    t0 = time.perf_counter()
    got = kernel(**inputs)
    print(f"kernel done in {time.perf_counter()-t0:.1f}s")
    rel = np.linalg.norm(got - exp) / (np.linalg.norm(exp) + 1e-30)
    mx = np.abs(got - exp).max()
    print(f"Relative error: {rel:.4e}   max-abs: {mx:.3e}  exp-scale: {np.abs(exp).max():.3f}")
